# revision 30
# baseline (speedup 1.0000x reference)
"""Multi-head attention (B=2, S=4096, D=512, H=8, causal + key-padding mask)
as a Bass/Tile kernel on 8 Trainium2 NeuronCores.

Sharding: core = (b, g) with b = core // 4, g = core % 4. Each core handles
one batch element and two heads (rows g*128:(g+1)*128 of Wq/Wk/Wv, and the
matching columns of Wo). The out-projection is row-sharded, so each core
returns a partial [S, D] output; the host sums the 4 partials per batch and
adds bo.

Per-core pipeline (all matmuls fp32r = full-rate single-pass fp32):
  1. x [S, D] -> xT [D, S] via PE transposes.
  2. qT, kT = W @ xT (heads stacked on partitions 0:64 / 64:128 so the two
     heads' score matmuls row-tile into the PE array concurrently);
     v in natural [skv, dv] layout with a ones column appended per head.
  3. For each head h, sq-chunk c (512 wide): for kv tile t (128 wide,
     t <= 4c+3 by causality): scores sT[skv, sq] = kT.T @ qT into PSUM;
     additive causal mask on diagonal-crossing tiles (DVE); exp via ACT
     (scale=1/8 folded in, key-padding log-mask as per-partition bias),
     batched two kv-tiles per ACT instruction; PV accumulation
     [v | 1].T @ p -> [65, 512] PSUM (row 64 = softmax denominator).
  4. reciprocal of the denominator, PE-broadcast to 64 partitions,
     DVE-normalize into vwT [dv, sq].
  5. out-proj: vwT.T @ WoT -> partial out [S, D], DMA straight from PSUM.
"""

import sys

if "/opt/trn_rl_repo" not in sys.path:
    sys.path.insert(0, "/opt/trn_rl_repo")

import numpy as np

import concourse.bass as bass
import concourse.mybir as mybir
import concourse.tile as tile
from concourse import bacc
from concourse.bass_utils import run_bass_kernel_spmd

F32 = mybir.dt.float32
F32R = mybir.dt.float32r
BF16 = mybir.dt.bfloat16

B, S, D, H = 2, 4096, 512, 8
DK = D // H            # 64 head dim
HPC = 2                # heads per core
DH = HPC * DK          # 128 features per core
NCORES = 8
SQC = 512              # sq chunk width (psum bank)
NSQC = S // SQC        # 8
KT = 128               # kv tile (partition dim)
NKT = S // KT          # 32
NF = D // 128          # 4 feature tiles
NST = S // 128         # 32 seq tiles
NEG = -1.0e9           # additive mask value (finite; exp(0.125*NEG) == 0.0)
CW = 2310 + DK + 2 * NKT  # packed-constants width (bf16 columns)


def _emit(nc, tc, trivial_mask, ctx):
    fp = ctx.enter_context(tc.tile_pool(name="persist", bufs=1))
    cp = ctx.enter_context(tc.tile_pool(name="consts", bufs=1))
    ldp = ctx.enter_context(tc.tile_pool(name="ld", bufs=3))
    vsp = ctx.enter_context(tc.tile_pool(name="vst", bufs=2))
    pp = ctx.enter_context(tc.tile_pool(name="pexp", bufs=3))
    smp = ctx.enter_context(tc.tile_pool(name="small", bufs=4))
    ps_sc = ctx.enter_context(tc.tile_pool(name="ps_sc", bufs=2, space="PSUM"))
    ps_pv = ctx.enter_context(tc.tile_pool(name="ps_pv", bufs=2, space="PSUM"))
    ps_ms = ctx.enter_context(tc.tile_pool(name="ps_ms", bufs=2, space="PSUM"))

    # DRAM I/O
    xtd = nc.dram_tensor("xT", [D, S], BF16, kind="ExternalInput").ap()
    cst = nc.dram_tensor("consts", [128, CW], BF16, kind="ExternalInput").ap()
    out = nc.dram_tensor("out", [S, D], F32, kind="ExternalOutput").ap()

    # ---- all constants arrive in ONE host-packed DMA (single semaphore) ----
    cs = cp.tile([128, CW], BF16, tag="cs")
    dma_c = nc.sync.dma_start(out=cs, in_=cst)
    wq_sb = cs[:, 0:512].rearrange("p (f m) -> p f m", f=NF)
    wk_sb = cs[:, 512:1024].rearrange("p (f m) -> p f m", f=NF)
    wv_sb = cs[:, 1024:1536].rearrange("p (f m) -> p f m", f=NF)
    wo_sb = cs[:, 1536:2048]
    dg_sb = cs[:, 2048:2176]
    id_sb = cs[:, 2176:2304]
    bq_sb = cs[:, 2304:2306].bitcast(F32)
    bk_sb = cs[:, 2306:2308].bitcast(F32)
    bv_sb = cs[:, 2308:2310].bitcast(F32)
    on_sb = cs[0:1, 2310 : 2310 + DK]
    kb_sb = cs[:, 2310 + DK : 2310 + DK + 2 * NKT].bitcast(F32)

    # ---- persistent intermediates ----
    xT = fp.tile([128, NF, S], BF16, tag="xT")       # x transposed, f-tiled
    qT = fp.tile([DH, S], BF16, tag="qT")            # heads on partitions
    kT = fp.tile([DH, S], BF16, tag="kT")
    vA = fp.tile([128, NKT, 2 * (DK + 1)], BF16, tag="vA")  # [skv, t, h*(65)]
    vwT = fp.tile([DH, S], BF16, tag="vwT")          # normalized context^T

    # ---- phase 1+2: load pre-transposed x, project q/k/v per sq chunk ----
    xtv = xtd.rearrange("(f p) s -> p f s", p=128)
    XCUTS = [0, 512, 1536, 2560, 4096]
    for qd in range(4):
        sl = slice(XCUTS[qd], XCUTS[qd + 1])
        nc.sync.dma_start(out=xT[:, :, sl], in_=xtv[:, :, sl])
    # Engines observe the input-DMA semaphores through tiny real
    # instructions so hot-loop consumers carry few waits (clock elision).
    wps = ps_ms.tile([1, 8], F32, tag="misc")
    nc.tensor.matmul(wps[0:1, 0:1], lhsT=cs[0:1, 0:1], rhs=cs[0:1, 0:1],
                     start=True, stop=True)
    wps2 = ps_ms.tile([1, 8], F32, tag="misc")
    nc.tensor.matmul(wps2[0:1, 0:1], lhsT=xT[0:1, 0, 0:1], rhs=xT[0:1, 0, 0:1],
                     start=True, stop=True)
    def absorb_quarter(qd):
        wpsq = ps_ms.tile([1, 8], F32, tag="misc")
        nc.tensor.matmul(wpsq[0:1, 0:1],
                         lhsT=xT[0:1, 0, XCUTS[qd] : XCUTS[qd] + 1],
                         rhs=xT[0:1, 0, XCUTS[qd] : XCUTS[qd] + 1],
                         start=True, stop=True)
    wsb = smp.tile([1, 8], F32, tag="wsb")
    nc.vector.tensor_copy(out=wsb[0:1, 0:1], in_=cs[0:1, 0:1])
    wsb2 = smp.tile([1, 8], F32, tag="wsb")
    nc.scalar.activation(out=wsb2[0:1, 0:1], in_=cs[0:1, 0:1],
                         func=mybir.ActivationFunctionType.Copy)
    tc.no_sync_barrier()
    # ones column of v_aug
    for h in range(HPC):
        nc.vector.memset(vA[:, :, h * (DK + 1) + DK : h * (DK + 1) + DK + 1], 1.0)
    def proj(c):
        sq = slice(c * SQC, (c + 1) * SQC)
        # q and k projections -> transposed layout [dh, sq]
        for w_sb, b_sb, dstT in ((wq_sb, bq_sb, qT), (wk_sb, bk_sb, kT)):
            prj = ps_sc.tile([DH, SQC], F32, tag="scores")
            for f in range(NF):
                nc.tensor.matmul(
                    prj,
                    lhsT=w_sb[:, f, :],
                    rhs=xT[:, f, sq],
                    start=(f == 0),
                    stop=(f == NF - 1),
                )
            nc.vector.tensor_scalar_add(dstT[:, sq], prj, b_sb)
        # v projection -> vT chunk, then transpose to natural layout
        prv = ps_sc.tile([DH, SQC], F32, tag="scores")
        for f in range(NF):
            nc.tensor.matmul(
                prv,
                lhsT=wv_sb[:, f, :],
                rhs=xT[:, f, sq],
                start=(f == 0),
                stop=(f == NF - 1),
            )
        vst = vsp.tile([DH, SQC], BF16, tag="vst")
        nc.vector.tensor_scalar_add(vst, prv, bv_sb)
        for j in range(4):
            t = 4 * c + j
            tps = ps_ms.tile([128, 128], BF16, tag="misc")
            nc.tensor.transpose(tps, vst[:, j * 128 : (j + 1) * 128], id_sb)
            for h in range(HPC):
                nc.vector.tensor_copy(
                    out=vA[:, t, h * (DK + 1) : h * (DK + 1) + DK],
                    in_=tps[:, h * DK : (h + 1) * DK],
                )

    def attn(h, c):
        hs = slice(h * DK, (h + 1) * DK)
        va = slice(h * (DK + 1), (h + 1) * (DK + 1))
        sq = slice(c * SQC, (c + 1) * SQC)
        nt = 4 * c + 4          # kv tiles 0..nt-1 (causal)
        pv = ps_pv.tile([DK + 1, SQC], F32, tag="pv")
        for tb in range((nt + 1) // 2):
            w = min(nt - tb * 2, 2)        # tiles in this batch (1|2)
            sc = ps_sc.tile([128, 2 * SQC], F32, tag="scores")
            for j in range(w):
                t = tb * 2 + j
                crossing = t >= 4 * c
                nc.tensor.matmul(
                    sc[:, j * SQC : (j + 1) * SQC],
                    lhsT=kT[hs, t * KT : (t + 1) * KT],
                    rhs=qT[hs, sq],
                    start=True,
                    stop=not crossing,
                )
                if crossing:  # accumulate causal mask on PE: += diag.T @ I
                    off = (t - 4 * c) * KT
                    blk = slice(j * SQC + off, j * SQC + off + KT)
                    nc.tensor.matmul(
                        sc[:, blk], lhsT=dg_sb, rhs=id_sb,
                        start=False, stop=True,
                    )
            p = pp.tile([128, 2 * SQC], BF16, tag="p")
            if trivial_mask:
                nc.scalar.activation(
                    out=p[:, : w * SQC],
                    in_=sc[:, : w * SQC],
                    func=mybir.ActivationFunctionType.Exp,
                    scale=0.125,
                )
            else:
                for j in range(w):
                    t = tb * 2 + j
                    nc.scalar.activation(
                        out=p[:, j * SQC : (j + 1) * SQC],
                        in_=sc[:, j * SQC : (j + 1) * SQC],
                        func=mybir.ActivationFunctionType.Exp,
                        bias=kb_sb[:, t : t + 1],
                        scale=0.125,
                    )
            for j in range(w):
                t = tb * 2 + j
                off = max(0, (t - 4 * c)) * KT
                nc.tensor.matmul(
                    pv[:, off:],
                    lhsT=vA[:, t, va],
                    rhs=p[:, j * SQC + off : (j + 1) * SQC],
                    start=(t == 0),
                    stop=(t == nt - 1),
                )
        # normalize: vw = pv[0:64] / pv[64]; 1/den broadcast to 64
        # partitions via PE with a bf16 hi+lo split (near-fp32 accuracy)
        rrf = smp.tile([1, SQC], F32, tag="rrf")
        nc.vector.reciprocal(rrf, pv[DK : DK + 1, :])
        rrh = smp.tile([1, SQC], BF16, tag="rrh")
        nc.vector.tensor_copy(out=rrh, in_=rrf)
        rrl = smp.tile([1, SQC], BF16, tag="rrl")
        nc.vector.tensor_sub(rrl, rrf, rrh)
        bc = ps_ms.tile([DK, SQC], F32, tag="misc")
        nc.tensor.matmul(bc, lhsT=on_sb, rhs=rrh, start=True, stop=False)
        nc.tensor.matmul(bc, lhsT=on_sb, rhs=rrl, start=False, stop=True)
        bc_sb = vsp.tile([DK, SQC], F32, tag="bcs")
        nc.vector.tensor_copy(out=bc_sb, in_=bc)
        nc.vector.tensor_mul(vwT[hs, sq], pv[0:DK, :], bc_sb)

    def outproj(m):
        po = ps_ms.tile([128, D], F32, tag="misc")
        nc.tensor.matmul(
            po,
            lhsT=vwT[:, m * 128 : (m + 1) * 128],
            rhs=wo_sb,
            start=True,
            stop=True,
        )
        ob = ldp.tile([128, D], F32, tag="ob")
        nc.vector.tensor_copy(out=ob, in_=po)
        nc.sync.dma_start(out=out[m * 128 : (m + 1) * 128, :], in_=ob)

    # fully interleaved: projection of chunk c immediately feeds attention on
    # both heads of chunk c, then the out-projection of its 4 seq tiles --
    # ACT starts exp within the first chunk instead of after all projections
    proj(0)
    for c in range(NSQC):
        attn(0, c)
        if c >= 1:
            for j in range(4):
                outproj(4 * (c - 1) + j)
        if c + 1 < NSQC:
            if c + 1 in (1, 3, 5):
                absorb_quarter((c + 1) // 2 + 1 if c + 1 == 5 else (c + 2) // 2)
            proj(c + 1)
        attn(1, c)
    for j in range(4):
        outproj(4 * (NSQC - 1) + j)


_CACHE = {}


def build_program(trivial_mask=True):
    key = trivial_mask
    if key not in _CACHE:
        from contextlib import ExitStack

        nc = bacc.Bacc("TRN2", target_bir_lowering=False, debug=False)
        with tile.TileContext(nc) as tc:
            with ExitStack() as ctx:
                ctx.enter_context(
                    nc.allow_low_precision(
                        reason="bf16 matmul operands (f32 psum accumulate)"
                    )
                )
                _emit(nc, tc, trivial_mask, ctx)
        nc.compile()
        _CACHE[key] = nc
    return _CACHE[key]


def make_in_maps(x, mask, Wq, bq, Wk, bk, Wv, bv, Wo, bo, trivial_mask=True):
    f = np.float32
    x = np.asarray(x, f)
    in_maps = []
    for core in range(NCORES):
        b, g = core // 4, core % 4
        sl = slice(g * DH, (g + 1) * DH)
        import ml_dtypes

        bf = ml_dtypes.bfloat16
        cw = np.zeros((128, CW), bf)

        def pack_w(wslice):
            # [m, (kt p)] -> [p, (kt m)]
            return (
                np.asarray(wslice, f)
                .reshape(DH, NF, 128)
                .transpose(2, 1, 0)
                .reshape(128, NF * DH)
            )

        def f32cols(a):
            # f32 [128, n] -> raw bits as bf16 [128, 2n]
            a = np.ascontiguousarray(np.asarray(a, f))
            return a.view(bf).reshape(128, -1)

        cw[:, 0:512] = pack_w(np.asarray(Wq, f)[sl, :]).astype(bf)
        cw[:, 512:1024] = pack_w(np.asarray(Wk, f)[sl, :]).astype(bf)
        cw[:, 1024:1536] = pack_w(np.asarray(Wv, f)[sl, :]).astype(bf)
        cw[:, 1536:2048] = np.asarray(Wo, f)[:, sl].T.astype(bf)
        cw[:, 2048:2176] = np.triu(np.full((KT, KT), NEG, f), 1).astype(bf)
        cw[:, 2176:2304] = np.eye(128, dtype=f).astype(bf)
        cw[:, 2304:2306] = f32cols(np.asarray(bq, f)[sl, None])
        cw[:, 2306:2308] = f32cols(np.asarray(bk, f)[sl, None])
        cw[:, 2308:2310] = f32cols(np.asarray(bv, f)[sl, None])
        cw[0, 2310 : 2310 + DK] = 1.0
        if not trivial_mask:
            kb = np.where(np.asarray(mask[b], bool), 0.0, NEG).astype(f)
            cw[:, 2310 + DK : 2310 + DK + 2 * NKT] = f32cols(
                kb.reshape(NKT, 128).T
            )
        m = {
            "xT": np.ascontiguousarray(x[b].T).astype(bf),
            "consts": cw,
        }
        in_maps.append(m)
    return in_maps


def kernel(x, mask, Wq, bq, Wk, bk, Wv, bv, Wo, bo, **run_kwargs):
    mask_np = np.asarray(mask, bool)
    trivial = bool(mask_np.all())
    nc = build_program(trivial)
    in_maps = make_in_maps(
        x, mask_np, Wq, bq, Wk, bk, Wv, bv, Wo, bo, trivial_mask=trivial
    )
    res = run_bass_kernel_spmd(nc, in_maps, core_ids=list(range(NCORES)), **run_kwargs)
    full = np.zeros((B, S, D), np.float32)
    for core in range(NCORES):
        full[core // 4] += res.results[core]["out"]
    full += np.asarray(bo, np.float32)
    kernel.last_results = res
    return full
